# revision 1
# baseline (speedup 1.0000x reference)
"""DeepAR (2-layer LSTM, H=512) Trainium2 Bass kernel.

Full-input contract: kernel(**inputs) takes the unsharded inputs from
setup_inputs() and returns the full [512, 64, 2] output.  Internally the
batch (512) is sharded 64-per-core across 8 NeuronCores (data parallel);
LSTM weights are replicated.

Device strategy (per core, B=64):
  - All weights resident in SBUF, streamed through the PE as the MOVING
    matmul operand each timestep (float32r -> 1 col/cycle).  Stationary
    operands are transposed activations [K<=128, 64].
  - Gates accumulate in PSUM as [64(batch), 512] chunks (i, g, f, o).
  - LSTM1 bias b1 (and the +1 forget bias) are folded in via a ones-row
    appended to the xy feature chunk; LSTM2 bias b2 is added on VectorE.
  - h is transposed back to [H, B] tiles via PE transpose each step.
  - Autoregressive decode feeds m = h2 @ Wm + bm back into the feature
    row in-place in SBUF; mean/disp outputs accumulate in the same tile.
"""
import sys

sys.path.insert(0, "/opt/trn_rl_repo")

import numpy as np

import concourse.bass as bass
import concourse.mybir as mybir
from concourse import bass_utils, tile

F32 = mybir.dt.float32
F32R = mybir.dt.float32r
Act = mybir.ActivationFunctionType

B_FULL, TP, TO, F, H = 512, 192, 128, 64, 512
NC = 8
B = B_FULL // NC            # 64 per core
G = 4 * H                   # 2048 gate width
NSLOT = TP + 1              # 193 feature slots (slot t feeds step t)
XCOLS = NSLOT * B           # 12352


def ts(i, n):
    return slice(i * n, (i + 1) * n)


def split_excess_waits(nc):
    """Walrus accepts only one sync-wait per hardware instruction. Hoist
    excess waits onto NoOps (same engine) inserted right before."""
    n = 0
    for f in nc.m.functions:
        for blk in f.blocks:
            out = []
            for inst in blk.instructions:
                si = inst.sync_info
                if si is not None and si.on_wait and len(si.on_wait) > 1:
                    waits = list(si.on_wait)
                    for j, w in enumerate(waits[:-1]):
                        nop = mybir.InstNoOp(
                            name=f"{inst.name}-wnop{j}", ins=[], outs=[])
                        nop.engine = inst.engine
                        nop.sync_info = mybir.SyncInfo(on_wait=[w], on_update=[])
                        out.append(nop)
                        n += 1
                    inst.sync_info = mybir.SyncInfo(
                        on_wait=[waits[-1]], on_update=list(si.on_update))
                out.append(inst)
            blk.instructions = out
    return n


def build_program(tp=TP, to=TO, split_waits=True):
    NSLOT_ = tp + 1
    XCOLS_ = NSLOT_ * B
    nc = bass.Bass("TRN2", target_bir_lowering=False, debug=False,
                   num_devices=NC)

    xyf_d = nc.dram_tensor("xyf_d", [66, XCOLS_], F32R, kind="ExternalInput").ap()
    w1c0_d = nc.dram_tensor("w1c0_d", [66, G], F32R, kind="ExternalInput").ap()
    w1h_d = nc.dram_tensor("w1h_d", [128, 4 * G], F32R, kind="ExternalInput").ap()
    w2_d = nc.dram_tensor("w2_d", [128, 8 * G], F32R, kind="ExternalInput").ap()
    wmd_d = nc.dram_tensor("wmd_d", [128, 4 * 64], F32R, kind="ExternalInput").ap()
    b2r_d = nc.dram_tensor("b2r_d", [64, G], F32, kind="ExternalInput").ap()
    bmd_d = nc.dram_tensor("bmd_d", [33, 1], F32, kind="ExternalInput").ap()
    id_d = nc.dram_tensor("id_d", [64, 64], F32, kind="ExternalInput").ap()
    out_d = nc.dram_tensor("out_d", [2, (tp - to) * B], F32,
                           kind="ExternalOutput").ap()

    with tile.TileContext(nc) as tc:
        with tc.sbuf_pool(name="const", bufs=1) as cp, \
             tc.sbuf_pool(name="work", bufs=1) as wp, \
             tc.psum_pool(name="ps", bufs=1) as pp:
            # ---- persistent tiles + input DMA ----
            xyf = cp.tile([66, XCOLS_], F32R, name="xyf")
            w1c0 = cp.tile([66, G], F32R, name="w1c0")
            w1h = cp.tile([128, 4 * G], F32R, name="w1h")
            w2 = cp.tile([128, 8 * G], F32R, name="w2")
            wmd = cp.tile([128, 4 * 64], F32R, name="wmd")
            b2r = cp.tile([64, G], F32, name="b2r")
            bmd = cp.tile([33, 1], F32, name="bmd")
            ident = cp.tile([64, 64], F32, name="ident")

            nc.sync.dma_start(xyf[:, :], xyf_d[:, :])
            nc.sync.dma_start(w1c0[:, :], w1c0_d[:, :])
            for k in range(4):
                nc.sync.dma_start(w1h[:, ts(k, G)], w1h_d[:, ts(k, G)])
            for k in range(8):
                nc.sync.dma_start(w2[:, ts(k, G)], w2_d[:, ts(k, G)])
            nc.sync.dma_start(wmd[:, :], wmd_d[:, :])
            nc.sync.dma_start(b2r[:, :], b2r_d[:, :])
            nc.sync.dma_start(bmd[:, :], bmd_d[:, :])
            nc.sync.dma_start(ident[:, :], id_d[:, :])

            # ---- state tiles ----
            c1 = cp.tile([64, H], F32, name="c1")
            c2 = cp.tile([64, H], F32, name="c2")
            nc.vector.memset(c1[:, :], 0.0)
            nc.vector.memset(c2[:, :], 0.0)

            h1T_prev = None
            h2T_prev = None
            g1p_pend = None

            def lstm_post(gp, c_state, b2_tile, htag):
                """gates psum chunks -> h [64, H] sbuf tile (fp32)."""
                if b2_tile is None:
                    i_s = wp.tile([64, H], F32, name=f"i{htag}", tag=f"i{htag}")
                    g_s = wp.tile([64, H], F32, name=f"g{htag}", tag=f"g{htag}")
                    f_s = wp.tile([64, H], F32, name=f"f{htag}", tag=f"f{htag}")
                    o_s = wp.tile([64, H], F32, name=f"o{htag}", tag=f"o{htag}")
                    nc.scalar.activation(i_s[:, :], gp[0][:, :], Act.Sigmoid)
                    nc.scalar.activation(g_s[:, :], gp[1][:, :], Act.Tanh)
                    nc.scalar.activation(f_s[:, :], gp[2][:, :], Act.Sigmoid)
                    nc.scalar.activation(o_s[:, :], gp[3][:, :], Act.Sigmoid)
                else:
                    # bias-add on DVE first (b2 replicated across partitions)
                    acts = []
                    fns = [Act.Sigmoid, Act.Tanh, Act.Sigmoid, Act.Sigmoid]
                    names = ["i", "g", "f", "o"]
                    for j in range(4):
                        pre = wp.tile([64, H], F32, name=f"pre{htag}_{j}",
                                      tag=f"pre{htag}", bufs=2)
                        nc.vector.tensor_add(pre[:, :], gp[j][:, :],
                                             b2_tile[:, ts(j, H)])
                        s = wp.tile([64, H], F32, name=f"{names[j]}{htag}",
                                    tag=f"{names[j]}{htag}")
                        nc.scalar.activation(s[:, :], pre[:, :], fns[j])
                        acts.append(s)
                    i_s, g_s, f_s, o_s = acts
                t1 = wp.tile([64, H], F32, name=f"t1{htag}", tag=f"t1{htag}")
                t2 = wp.tile([64, H], F32, name=f"t2{htag}", tag=f"t2{htag}")
                nc.vector.tensor_mul(t1[:, :], i_s[:, :], g_s[:, :])
                nc.vector.tensor_mul(t2[:, :], f_s[:, :], c_state[:, :])
                nc.vector.tensor_add(c_state[:, :], t1[:, :], t2[:, :])
                tc_s = wp.tile([64, H], F32, name=f"tc{htag}", tag=f"t1{htag}",
                               bufs=1)
                nc.scalar.activation(tc_s[:, :], c_state[:, :], Act.Tanh)
                h = wp.tile([64, H], F32, name=f"h{htag}", tag=f"h{htag}")
                nc.vector.tensor_mul(h[:, :], o_s[:, :], tc_s[:, :])
                return h

            def transpose_h(h, htag):
                trp = pp.tile([128, 256], F32, name=f"tr{htag}", tag="small",
                              bufs=1)
                for kk in range(4):
                    nc.tensor.transpose(trp[:, ts(kk, 64)],
                                        h[:, ts(kk, 128)], ident[:, :])
                hT = wp.tile([128, 256], F32R, name=f"hT{htag}",
                             tag=f"hT{htag}", bufs=2)
                nc.vector.tensor_copy(hT[:, 0:128], trp[:, 0:128])
                nc.vector.tensor_copy(hT[:, 128:256], trp[:, 128:256])
                return hT

            for t in range(tp):
                first = t == 0
                # --- phase A: finish L1(t) gates with the xy chunk ---
                if g1p_pend is None:
                    g1p = [pp.tile([64, H], F32, name=f"g1p{j}", tag="g1",
                                   bufs=4) for j in range(4)]
                else:
                    g1p = g1p_pend
                # --- phase B first: L2(t) h2-part (no dependence on m) ---
                g2p = [pp.tile([64, H], F32, name=f"g2p{j}", tag="g2",
                               bufs=3) for j in range(4)]
                if not first:
                    for j in range(4):
                        for k in range(4):
                            nc.tensor.matmul(
                                g2p[j][:, :], h2T_prev[:, ts(k, 64)],
                                w2[:, (4 + k) * G + j * H:(4 + k) * G + (j + 1) * H],
                                start=(k == 0), stop=False,
                                skip_group_check=True)
                # --- phase A: finish L1(t) gates with the xy chunk (AR: waits m) ---
                for j in range(4):
                    nc.tensor.matmul(g1p[j][:, :], xyf[0:66, ts(t, 64)],
                                     w1c0[:, ts(j, H)], start=first,
                                     stop=True, skip_group_check=True)
                # --- phase C: L1 post + h1 transpose ---
                h1 = lstm_post(g1p, c1, None, "1")
                h1T = transpose_h(h1, "1")
                # --- phase D: L2(t) h1-part ---
                for j in range(4):
                    for k in range(4):
                        nc.tensor.matmul(
                            g2p[j][:, :], h1T[:, ts(k, 64)],
                            w2[:, k * G + j * H:k * G + (j + 1) * H],
                            start=(first and k == 0), stop=(k == 3),
                            skip_group_check=True)
                # --- phase E: L1(t+1) h-part (pipelined ahead) ---
                if t < tp - 1:
                    g1p_pend = [pp.tile([64, H], F32, name=f"g1pn{j}",
                                        tag="g1", bufs=4) for j in range(4)]
                    for j in range(4):
                        for k in range(4):
                            nc.tensor.matmul(
                                g1p_pend[j][:, :], h1T[:, ts(k, 64)],
                                w1h[:, k * G + j * H:k * G + (j + 1) * H],
                                start=(k == 0), stop=False,
                                skip_group_check=True)
                else:
                    g1p_pend = None
                # --- phase F: L2 post + h2 transpose ---
                h2 = lstm_post(g2p, c2, b2r, "2")
                h2T = transpose_h(h2, "2")
                # --- phase G: m/d head (AR feedback + outputs) ---
                if t >= to - 1:
                    mdp = pp.tile([64, 64], F32, name="mdp", tag="small",
                                  bufs=1)
                    for k in range(4):
                        nc.tensor.matmul(mdp[:, :], wmd[:, ts(k, 64)],
                                         h2T[:, ts(k, 64)], start=(k == 0),
                                         stop=(k == 3),
                                         skip_group_check=True)
                    # m -> feature row 0, slot t+1 (f32r rounding on write)
                    nc.scalar.activation(xyf[0:1, ts(t + 1, 64)],
                                         mdp[0:1, :], Act.Identity,
                                         bias=bmd[0:1, 0:1], scale=1.0)
                    if t >= to:
                        # d -> row 64 (ones/d row), slot t (already consumed)
                        nc.scalar.activation(xyf[64:65, ts(t, 64)],
                                             mdp[32:33, :], Act.Identity,
                                             bias=bmd[32:33, 0:1], scale=1.0)
                h1T_prev, h2T_prev = h1T, h2T

            # ---- outputs: mean row = slots TO+1..TP, disp row = slots TO..TP-1
            nc.sync.dma_start(out_d[0:1, :],
                              xyf[0:1, (to + 1) * B:(tp + 1) * B].bitcast(F32))
            nc.sync.dma_start(out_d[1:2, :],
                              xyf[64:65, to * B:tp * B].bitcast(F32))

    n = split_excess_waits(nc) if split_waits else 0
    return nc, n


_CACHE = {}


def _get_program():
    if "nc" not in _CACHE:
        _CACHE["nc"] = build_program()[0]
    return _CACHE["nc"]


def make_core_inputs(x, y, W1, b1, W2, b2, Wm, bm, Wd, bd, tp=TP, to=TO):
    """Host-side prep: returns (in_maps list of 8 dicts, scale [512])."""
    NSLOT_ = tp + 1
    XCOLS_ = NSLOT_ * B
    x = np.asarray(x, np.float32)
    y = np.asarray(y, np.float32)
    W1 = np.asarray(W1, np.float32)
    b1 = np.asarray(b1, np.float32)
    W2 = np.asarray(W2, np.float32)
    b2 = np.asarray(b2, np.float32)
    Wm = np.asarray(Wm, np.float32)
    bm = np.asarray(bm, np.float32)
    Wd = np.asarray(Wd, np.float32)
    bd = np.asarray(bd, np.float32)

    scale = 1.0 + np.mean(y[:, 0:to, 0], axis=1)       # [512]
    y_sc = y[:, 0:to, 0] / scale[:, None]              # [512, to]

    b1a = b1.copy()
    b1a[2 * H:3 * H] += 1.0                             # forget-gate +1
    b2a = b2.copy()
    b2a[2 * H:3 * H] += 1.0

    # row layout: 0 = y/m, 1:64 = x[0:63], 64 = ones/bias (disp storage),
    # 65 = x[63]  (rows 0 and 64 must sit at legal engine partition bases)
    w1c0 = np.empty((66, G), np.float32)
    w1c0[0] = W1[F]                                     # y/m weight row
    w1c0[1:64] = W1[0:F - 1]                            # x weight rows 0..62
    w1c0[64] = b1a                                      # bias row (ones input)
    w1c0[65] = W1[F - 1]                                # x weight row 63

    w1h = np.ascontiguousarray(
        W1[F + 1:].reshape(4, 128, G).transpose(1, 0, 2).reshape(128, 4 * G))
    w2 = np.ascontiguousarray(
        W2.reshape(8, 128, G).transpose(1, 0, 2).reshape(128, 8 * G))

    wmd = np.zeros((128, 4, 64), np.float32)
    wmd[:, :, 0] = Wm[:, 0].reshape(4, 128).T
    wmd[:, :, 32] = Wd[:, 0].reshape(4, 128).T
    wmd = np.ascontiguousarray(wmd.reshape(128, 4 * 64))

    b2rep = np.ascontiguousarray(np.broadcast_to(b2a, (64, G)))
    bmd = np.zeros((33, 1), np.float32)
    bmd[0, 0] = bm[0]
    bmd[32, 0] = bd[0]
    identity = np.eye(64, dtype=np.float32)

    in_maps = []
    for c in range(NC):
        bs = slice(c * B, (c + 1) * B)
        xyf = np.zeros((66, NSLOT_, B), np.float32)
        xyf[0, 1:to, :] = y_sc[bs, 0:to - 1].T          # shifted y feed
        xt = x[bs].transpose(2, 1, 0)                   # [f, t, b]
        xyf[1:64, 0:tp, :] = xt[0:F - 1]                # x rows 0..62
        xyf[65, 0:tp, :] = xt[F - 1]                    # x row 63
        xyf[64, :, :] = 1.0                             # ones / bias row
        in_maps.append({
            "xyf_d": np.ascontiguousarray(xyf.reshape(66, XCOLS_)),
            "w1c0_d": w1c0, "w1h_d": w1h, "w2_d": w2, "wmd_d": wmd,
            "b2r_d": b2rep, "bmd_d": bmd, "id_d": identity,
        })
    return in_maps, scale


def postprocess(results, scale, tp=TP, to=TO):
    """results: list of 8 dicts with out_d [2, (tp-to)*64] -> [512, tp-to, 2]."""
    out = np.empty((B_FULL, tp - to, 2), np.float32)
    for c in range(NC):
        r = results[c]["out_d"]
        mean_tb = r[0].reshape(tp - to, B)              # [t, b]
        dpre_tb = r[1].reshape(tp - to, B)
        bs = slice(c * B, (c + 1) * B)
        sc = scale[bs]
        out[bs, :, 0] = (mean_tb * sc[None, :]).T
        disp = np.logaddexp(dpre_tb, 0.0)               # softplus
        out[bs, :, 1] = (disp * np.sqrt(sc)[None, :]).T
    return out


def kernel(x, y, W1, b1, W2, b2, Wm, bm, Wd, bd):
    in_maps, scale = make_core_inputs(x, y, W1, b1, W2, b2, Wm, bm, Wd, bd)
    nc = _get_program()
    res = bass_utils.run_bass_kernel_spmd(nc, in_maps, core_ids=list(range(NC)))
    return postprocess(res.results, scale)



# revision 5
# speedup vs baseline: 2.7669x; 2.7669x over previous
"""DeepAR (2-layer LSTM, H=512) Trainium2 Bass kernel — v2.

Full-input contract: kernel(**inputs) takes the unsharded inputs from
setup_inputs() and returns the full [512, 64, 2] output.  Internally the
batch (512) is sharded 64-per-core across 8 NeuronCores (data parallel);
LSTM weights are replicated.

v2 changes over v1:
  - bf16 matmul operands (weights, features, transposed h); fp32 psum.
  - Phase A (L1 xy finisher) emitted FIRST each step so the h1 recurrence
    no longer chains through L2post+B. A is split into N=256 halves so
    the LSTM post can start per-half.
  - D/E are k-chunk-major so contraction chunks start as soon as each
    transposed h half lands; D's k=3 finisher is split into N=256 halves
    so L2post pipelines the same way.
  - L2 bias b2 enters PSUM via K=1 ones-row matmuls (start=True) instead
    of four DVE adds on the critical chain.
  - LSTM posts run per 256-col half: ACT reads gate psum directly,
    DVE does the c/h updates, ACT copies the transposed h into bf16.
  - Decode: m/d head outputs are copied from psum into an SBUF staging
    tile (host adds bm/bd); only the m feedback write touches xyf.
"""
import sys

sys.path.insert(0, "/opt/trn_rl_repo")

import numpy as np

import concourse.bass as bass
import concourse.mybir as mybir
from concourse import bass_utils, tile

F32 = mybir.dt.float32
BF16 = mybir.dt.bfloat16
Act = mybir.ActivationFunctionType

B_FULL, TP, TO, F, H = 512, 192, 128, 64, 512
NC = 8
B = B_FULL // NC            # 64 per core
G = 4 * H                   # 2048 gate width
NSLOT = TP + 1              # 193 feature slots (slot t feeds step t)
XCOLS = NSLOT * B           # 12352
HH = H // 2                 # 256: half of hidden


def ts(i, n):
    return slice(i * n, (i + 1) * n)


def split_excess_waits(nc):
    """Walrus accepts only one sync-wait per hardware instruction. Hoist
    excess waits onto NoOps (same engine) inserted right before."""
    n = 0
    for f in nc.m.functions:
        for blk in f.blocks:
            out = []
            for inst in blk.instructions:
                si = inst.sync_info
                if si is not None and si.on_wait and len(si.on_wait) > 1:
                    waits = list(si.on_wait)
                    for j, w in enumerate(waits[:-1]):
                        nop = mybir.InstNoOp(
                            name=f"{inst.name}-wnop{j}", ins=[], outs=[])
                        nop.engine = inst.engine
                        nop.sync_info = mybir.SyncInfo(on_wait=[w], on_update=[])
                        out.append(nop)
                        n += 1
                    inst.sync_info = mybir.SyncInfo(
                        on_wait=[waits[-1]], on_update=list(si.on_update))
                out.append(inst)
            blk.instructions = out
    return n


def build_program(tp=TP, to=TO, split_waits=True):
    NSLOT_ = tp + 1
    XCOLS_ = NSLOT_ * B
    TD = tp - to                # decode steps (64)
    nc = bass.Bass("TRN2", target_bir_lowering=False, debug=False,
                   num_devices=NC)

    xyf_d = nc.dram_tensor("xyf_d", [66, XCOLS_], BF16, kind="ExternalInput").ap()
    w1c0_d = nc.dram_tensor("w1c0_d", [66, G], BF16, kind="ExternalInput").ap()
    w1h_d = nc.dram_tensor("w1h_d", [128, 4 * G], BF16, kind="ExternalInput").ap()
    w2_d = nc.dram_tensor("w2_d", [128, 8 * G], BF16, kind="ExternalInput").ap()
    wmd_d = nc.dram_tensor("wmd_d", [128, 4 * 64], BF16, kind="ExternalInput").ap()
    b2r_d = nc.dram_tensor("b2r_d", [1, G], BF16, kind="ExternalInput").ap()
    bmd_d = nc.dram_tensor("bmd_d", [1, 1], F32, kind="ExternalInput").ap()
    id_d = nc.dram_tensor("id_d", [64, 64], F32, kind="ExternalInput").ap()
    outmd_d = nc.dram_tensor("outmd_d", [1, TD * 128], F32,
                             kind="ExternalOutput").ap()

    with tile.TileContext(nc) as tc:
        with tc.sbuf_pool(name="const", bufs=1) as cp, \
             tc.sbuf_pool(name="work", bufs=1) as wp, \
             tc.psum_pool(name="ps", bufs=1) as pp:
            # ---- persistent tiles + input DMA ----
            xyf = cp.tile([66, XCOLS_], BF16, name="xyf")
            w1c0 = cp.tile([66, G], BF16, name="w1c0")
            w1h = cp.tile([128, 4 * G], BF16, name="w1h")
            w2 = cp.tile([128, 8 * G], BF16, name="w2")
            wmd = cp.tile([128, 4 * 64], BF16, name="wmd")
            b2r = cp.tile([1, G], BF16, name="b2r")
            bmd = cp.tile([1, 1], F32, name="bmd")
            ident = cp.tile([64, 64], F32, name="ident")
            outmd = cp.tile([1, TD * 128], F32, name="outmd")
            ones = cp.tile([1, 64], BF16, name="ones")
            nc.vector.memset(ones[:, :], 1.0)

            nc.sync.dma_start(xyf[:, :], xyf_d[:, :])
            nc.sync.dma_start(w1c0[:, :], w1c0_d[:, :])
            for k in range(4):
                nc.sync.dma_start(w1h[:, ts(k, G)], w1h_d[:, ts(k, G)])
            for k in range(8):
                nc.sync.dma_start(w2[:, ts(k, G)], w2_d[:, ts(k, G)])
            nc.sync.dma_start(wmd[:, :], wmd_d[:, :])
            nc.sync.dma_start(b2r[:, :], b2r_d[:, :])
            nc.sync.dma_start(bmd[:, :], bmd_d[:, :])
            nc.sync.dma_start(ident[:, :], id_d[:, :])

            # ---- state tiles ----
            c1 = cp.tile([64, H], F32, name="c1")
            c2 = cp.tile([64, H], F32, name="c2")
            nc.vector.memset(c1[:, :], 0.0)
            nc.vector.memset(c2[:, :], 0.0)

            def post_half(gp, c_state, h, half, htag):
                """LSTM post for columns [half*HH:(half+1)*HH]."""
                sl = ts(half, HH)
                i_s = wp.tile([64, HH], F32, name=f"i{htag}", tag=f"i{htag}",
                              bufs=2)
                g_s = wp.tile([64, HH], F32, name=f"g{htag}", tag=f"g{htag}",
                              bufs=2)
                f_s = wp.tile([64, HH], F32, name=f"f{htag}", tag=f"f{htag}",
                              bufs=2)
                o_s = wp.tile([64, HH], F32, name=f"o{htag}", tag=f"o{htag}",
                              bufs=2)
                nc.scalar.activation(i_s[:, :], gp[0][:, sl], Act.Sigmoid)
                nc.scalar.activation(g_s[:, :], gp[1][:, sl], Act.Tanh)
                nc.scalar.activation(f_s[:, :], gp[2][:, sl], Act.Sigmoid)
                nc.scalar.activation(o_s[:, :], gp[3][:, sl], Act.Sigmoid)
                t1 = wp.tile([64, HH], F32, name=f"t1{htag}", tag=f"t1{htag}",
                             bufs=2)
                t2 = wp.tile([64, HH], F32, name=f"t2{htag}", tag=f"t2{htag}",
                             bufs=2)
                nc.vector.tensor_mul(t1[:, :], i_s[:, :], g_s[:, :])
                nc.vector.tensor_mul(t2[:, :], f_s[:, :], c_state[:, sl])
                nc.vector.tensor_add(c_state[:, sl], t1[:, :], t2[:, :])
                tc_s = wp.tile([64, HH], F32, name=f"tc{htag}",
                               tag=f"tc{htag}", bufs=2)
                nc.scalar.activation(tc_s[:, :], c_state[:, sl], Act.Tanh)
                nc.vector.tensor_mul(h[:, sl], o_s[:, :], tc_s[:, :])

            def transpose_half(trp, h, hT, half):
                """h[:, half] -> trp psum -> hT bf16 chunk cols."""
                for kk in (2 * half, 2 * half + 1):
                    nc.tensor.transpose(trp[:, ts(kk, 64)],
                                        h[:, ts(kk, 128)], ident[:, :])
                nc.scalar.activation(hT[:, ts(half, 128)],
                                     trp[:, ts(half, 128)], Act.Identity)

            h1T_prev = None
            h2T_prev = None
            g1p_pend = None

            for t in range(tp):
                first = t == 0
                dec = t >= to - 1      # head/m-feedback steps
                # --- A: L1 xy finisher, N=256 halves (waits m-write in AR) ---
                if g1p_pend is None:
                    g1p = [pp.tile([64, H], F32, name=f"g1p{j}", tag="g1",
                                   bufs=4) for j in range(4)]
                else:
                    g1p = g1p_pend
                for half in range(2):
                    for j in range(4):
                        nc.tensor.matmul(
                            g1p[j][:, ts(half, HH)], xyf[0:66, ts(t, 64)],
                            w1c0[:, j * H + half * HH:j * H + (half + 1) * HH],
                            start=first, stop=True, skip_group_check=True)
                # --- g2 group: bias (start) + B: h2 part, k-major ---
                g2p = [pp.tile([64, H], F32, name=f"g2p{j}", tag="g2",
                               bufs=3) for j in range(4)]
                for j in range(4):
                    nc.tensor.matmul(g2p[j][:, :], ones[:, :],
                                     b2r[:, ts(j, H)], start=True, stop=False,
                                     skip_group_check=True)
                if not first:
                    for k in range(4):
                        for j in range(4):
                            nc.tensor.matmul(
                                g2p[j][:, :], h2T_prev[:, ts(k, 64)],
                                w2[:, (4 + k) * G + j * H:(4 + k) * G + (j + 1) * H],
                                start=False, stop=False,
                                skip_group_check=True)
                # --- L1 post + transpose, per half ---
                h1 = wp.tile([64, H], F32, name="h1", tag="h1", bufs=2)
                h1T = wp.tile([128, 256], BF16, name="h1T", tag="h1T", bufs=2)
                trp1 = pp.tile([128, 256], F32, name="trp1", tag="small",
                               bufs=1)
                for half in range(2):
                    post_half(g1p, c1, h1, half, "1")
                    transpose_half(trp1, h1, h1T, half)
                # --- D: L2 h1 part, k-major; k=3 finisher split in halves ---
                for k in range(3):
                    for j in range(4):
                        nc.tensor.matmul(
                            g2p[j][:, :], h1T[:, ts(k, 64)],
                            w2[:, k * G + j * H:k * G + (j + 1) * H],
                            start=False, stop=False, skip_group_check=True)
                for half in range(2):
                    for j in range(4):
                        nc.tensor.matmul(
                            g2p[j][:, ts(half, HH)], h1T[:, ts(3, 64)],
                            w2[:, 3 * G + j * H + half * HH:
                               3 * G + j * H + (half + 1) * HH],
                            start=False, stop=True, skip_group_check=True)
                # --- E: L1 h part for t+1, k-major (pipelined ahead) ---
                if t < tp - 1:
                    g1p_pend = [pp.tile([64, H], F32, name=f"g1pn{j}",
                                        tag="g1", bufs=4) for j in range(4)]
                    for k in range(4):
                        for j in range(4):
                            nc.tensor.matmul(
                                g1p_pend[j][:, :], h1T[:, ts(k, 64)],
                                w1h[:, k * G + j * H:k * G + (j + 1) * H],
                                start=(k == 0), stop=False,
                                skip_group_check=True)
                else:
                    g1p_pend = None
                # --- L2 post + transpose, per half ---
                h2 = wp.tile([64, H], F32, name="h2", tag="h2", bufs=2)
                h2T = wp.tile([128, 256], BF16, name="h2T", tag="h2T", bufs=2)
                trp2 = pp.tile([128, 256], F32, name="trp2", tag="small",
                               bufs=1)
                for half in range(2):
                    post_half(g2p, c2, h2, half, "2")
                    transpose_half(trp2, h2, h2T, half)
                # --- G: m/d head (AR feedback + staged outputs) ---
                if dec:
                    mdp = pp.tile([64, 64], F32, name="mdp", tag="small",
                                  bufs=1)
                    for k in range(4):
                        nc.tensor.matmul(mdp[:, :], wmd[:, ts(k, 64)],
                                         h2T[:, ts(k, 64)], start=(k == 0),
                                         stop=(k == 3),
                                         skip_group_check=True)
                    if t < tp - 1:
                        # m feedback -> feature row 0, slot t+1 (bf16)
                        nc.scalar.activation(xyf[0:1, ts(t + 1, 64)],
                                             mdp[0:1, :], Act.Identity,
                                             bias=bmd[0:1, 0:1], scale=1.0)
                    if t >= to:
                        s = t - to
                        # raw m/d rows -> staging tile (host adds bm/bd)
                        nc.scalar.activation(outmd[0:1, s * 128:s * 128 + 64],
                                             mdp[0:1, :], Act.Identity)
                        nc.scalar.activation(
                            outmd[0:1, s * 128 + 64:s * 128 + 128],
                            mdp[32:33, :], Act.Identity)
                h1T_prev, h2T_prev = h1T, h2T

            nc.sync.dma_start(outmd_d[:, :], outmd[:, :])

    n = split_excess_waits(nc) if split_waits else 0
    return nc, n


_CACHE = {}


def _get_program():
    if "nc" not in _CACHE:
        _CACHE["nc"] = build_program()[0]
    return _CACHE["nc"]


def make_core_inputs(x, y, W1, b1, W2, b2, Wm, bm, Wd, bd, tp=TP, to=TO):
    """Host-side prep: returns (in_maps list of 8 dicts, scale [512])."""
    import ml_dtypes
    bf16 = ml_dtypes.bfloat16
    NSLOT_ = tp + 1
    XCOLS_ = NSLOT_ * B
    x = np.asarray(x, np.float32)
    y = np.asarray(y, np.float32)
    W1 = np.asarray(W1, np.float32)
    b1 = np.asarray(b1, np.float32)
    W2 = np.asarray(W2, np.float32)
    b2 = np.asarray(b2, np.float32)
    Wm = np.asarray(Wm, np.float32)
    bm = np.asarray(bm, np.float32)
    Wd = np.asarray(Wd, np.float32)
    bd = np.asarray(bd, np.float32)

    scale = 1.0 + np.mean(y[:, 0:to, 0], axis=1)       # [512]
    y_sc = y[:, 0:to, 0] / scale[:, None]              # [512, to]

    b1a = b1.copy()
    b1a[2 * H:3 * H] += 1.0                             # forget-gate +1
    b2a = b2.copy()
    b2a[2 * H:3 * H] += 1.0

    # row layout: 0 = y/m, 1:64 = x[0:63], 64 = ones (bias), 65 = x[63]
    w1c0 = np.empty((66, G), np.float32)
    w1c0[0] = W1[F]                                     # y/m weight row
    w1c0[1:64] = W1[0:F - 1]                            # x weight rows 0..62
    w1c0[64] = b1a                                      # bias row (ones input)
    w1c0[65] = W1[F - 1]                                # x weight row 63

    w1h = np.ascontiguousarray(
        W1[F + 1:].reshape(4, 128, G).transpose(1, 0, 2).reshape(128, 4 * G))
    w2 = np.ascontiguousarray(
        W2.reshape(8, 128, G).transpose(1, 0, 2).reshape(128, 8 * G))

    wmd = np.zeros((128, 4, 64), np.float32)
    wmd[:, :, 0] = Wm[:, 0].reshape(4, 128).T
    wmd[:, :, 32] = Wd[:, 0].reshape(4, 128).T
    wmd = np.ascontiguousarray(wmd.reshape(128, 4 * 64))

    b2row = np.ascontiguousarray(b2a.reshape(1, G))
    bmd = np.asarray(bm, np.float32).reshape(1, 1)
    identity = np.eye(64, dtype=np.float32)

    in_maps = []
    for c in range(NC):
        bs = slice(c * B, (c + 1) * B)
        xyf = np.zeros((66, NSLOT_, B), np.float32)
        xyf[0, 1:to, :] = y_sc[bs, 0:to - 1].T          # shifted y feed
        xt = x[bs].transpose(2, 1, 0)                   # [f, t, b]
        xyf[1:64, 0:tp, :] = xt[0:F - 1]                # x rows 0..62
        xyf[65, 0:tp, :] = xt[F - 1]                    # x row 63
        xyf[64, :, :] = 1.0                             # ones / bias row
        in_maps.append({
            "xyf_d": np.ascontiguousarray(
                xyf.reshape(66, XCOLS_)).astype(bf16),
            "w1c0_d": w1c0.astype(bf16), "w1h_d": w1h.astype(bf16),
            "w2_d": w2.astype(bf16), "wmd_d": wmd.astype(bf16),
            "b2r_d": b2row.astype(bf16), "bmd_d": bmd, "id_d": identity,
        })
    return in_maps, scale


def postprocess(results, scale, bm, bd, tp=TP, to=TO):
    """results: list of 8 dicts with outmd_d [tp-to, 128] -> [512, tp-to, 2]."""
    bm = float(np.asarray(bm).reshape(-1)[0])
    bd = float(np.asarray(bd).reshape(-1)[0])
    out = np.empty((B_FULL, tp - to, 2), np.float32)
    for c in range(NC):
        r = results[c]["outmd_d"].reshape(tp - to, 128)
        mean_tb = r[:, 0:64] + bm                       # [t, b]
        dpre_tb = r[:, 64:128] + bd
        bs = slice(c * B, (c + 1) * B)
        sc = scale[bs]
        out[bs, :, 0] = (mean_tb * sc[None, :]).T
        disp = np.logaddexp(dpre_tb, 0.0)               # softplus
        out[bs, :, 1] = (disp * np.sqrt(sc)[None, :]).T
    return out


def kernel(x, y, W1, b1, W2, b2, Wm, bm, Wd, bd):
    in_maps, scale = make_core_inputs(x, y, W1, b1, W2, b2, Wm, bm, Wd, bd)
    nc = _get_program()
    res = bass_utils.run_bass_kernel_spmd(nc, in_maps, core_ids=list(range(NC)))
    return postprocess(res.results, scale, bm, bd)


# revision 10
# speedup vs baseline: 2.8250x; 1.0210x over previous
"""DeepAR (2-layer LSTM, H=512) Trainium2 Bass kernel — v2.

Full-input contract: kernel(**inputs) takes the unsharded inputs from
setup_inputs() and returns the full [512, 64, 2] output.  Internally the
batch (512) is sharded 64-per-core across 8 NeuronCores (data parallel);
LSTM weights are replicated.

v2 changes over v1:
  - bf16 matmul operands (weights, features, transposed h); fp32 psum.
  - Phase A (L1 xy finisher) emitted FIRST each step so the h1 recurrence
    no longer chains through L2post+B. A is split into N=256 halves so
    the LSTM post can start per-half.
  - D/E are k-chunk-major so contraction chunks start as soon as each
    transposed h half lands; D's k=3 finisher is split into N=256 halves
    so L2post pipelines the same way.
  - L2 bias b2 enters PSUM via K=1 ones-row matmuls (start=True) instead
    of four DVE adds on the critical chain.
  - LSTM posts run per 256-col half: ACT reads gate psum directly,
    DVE does the c/h updates, ACT copies the transposed h into bf16.
  - Decode: m/d head outputs are copied from psum into an SBUF staging
    tile (host adds bm/bd); only the m feedback write touches xyf.
"""
import sys

sys.path.insert(0, "/opt/trn_rl_repo")

import numpy as np

import concourse.bass as bass
import concourse.mybir as mybir
from concourse import bass_utils, tile

F32 = mybir.dt.float32
BF16 = mybir.dt.bfloat16
Act = mybir.ActivationFunctionType

B_FULL, TP, TO, F, H = 512, 192, 128, 64, 512
NC = 8
B = B_FULL // NC            # 64 per core
G = 4 * H                   # 2048 gate width
NSLOT = TP + 1              # 193 feature slots (slot t feeds step t)
XCOLS = NSLOT * B           # 12352
HH = H // 2                 # 256: half of hidden


def ts(i, n):
    return slice(i * n, (i + 1) * n)


def split_excess_waits(nc):
    """Walrus accepts only one sync-wait per hardware instruction. Hoist
    excess waits onto NoOps (same engine) inserted right before."""
    n = 0
    for f in nc.m.functions:
        for blk in f.blocks:
            out = []
            for inst in blk.instructions:
                si = inst.sync_info
                if si is not None and si.on_wait and len(si.on_wait) > 1:
                    waits = list(si.on_wait)
                    for j, w in enumerate(waits[:-1]):
                        nop = mybir.InstNoOp(
                            name=f"{inst.name}-wnop{j}", ins=[], outs=[])
                        nop.engine = inst.engine
                        nop.sync_info = mybir.SyncInfo(on_wait=[w], on_update=[])
                        out.append(nop)
                        n += 1
                    inst.sync_info = mybir.SyncInfo(
                        on_wait=[waits[-1]], on_update=list(si.on_update))
                out.append(inst)
            blk.instructions = out
    return n


def drop_redundant_ldweights(nc):
    """Remove InstLdweights that reload the stationary operand already in
    the PE array (identical AP as the previous retained load, no transpose
    in between). Waits/updates on dropped loads survive on a PE NoOp."""
    n = 0
    for f in nc.m.functions:
        for blk in f.blocks:
            out = []
            last_key = None
            for inst in blk.instructions:
                if isinstance(inst, mybir.InstLdweights):
                    w = inst.ins[0]
                    key = (getattr(w, "memref", None), w.offset, str(w.ap),
                           str(w.dtype), str(inst.perf_mode),
                           str(inst.tile_position))
                    if key == last_key:
                        si = inst.sync_info
                        if si is not None and (si.on_wait or si.on_update):
                            nop = mybir.InstNoOp(
                                name=f"{inst.name}-ldwnop", ins=[], outs=[])
                            nop.engine = inst.engine
                            nop.sync_info = si
                            out.append(nop)
                        n += 1
                        continue
                    last_key = key
                elif isinstance(inst, mybir.InstMatmult):
                    if inst.is_transpose:
                        last_key = None
                out.append(inst)
            blk.instructions = out
    return n


def build_program(tp=TP, to=TO, split_waits=True, noload=True):
    NSLOT_ = tp + 1
    XCOLS_ = NSLOT_ * B
    TD = tp - to                # decode steps (64)
    nc = bass.Bass("TRN2", target_bir_lowering=False, debug=False,
                   num_devices=NC)

    xyf_d = nc.dram_tensor("xyf_d", [66, XCOLS_], BF16, kind="ExternalInput").ap()
    w1c0_d = nc.dram_tensor("w1c0_d", [66, G], BF16, kind="ExternalInput").ap()
    w1h_d = nc.dram_tensor("w1h_d", [128, 4 * G], BF16, kind="ExternalInput").ap()
    w2_d = nc.dram_tensor("w2_d", [128, 8 * G], BF16, kind="ExternalInput").ap()
    wmd_d = nc.dram_tensor("wmd_d", [128, 4 * 64], BF16, kind="ExternalInput").ap()
    b2r_d = nc.dram_tensor("b2r_d", [1, G], BF16, kind="ExternalInput").ap()
    bmd_d = nc.dram_tensor("bmd_d", [1, 1], F32, kind="ExternalInput").ap()
    id_d = nc.dram_tensor("id_d", [64, 64], F32, kind="ExternalInput").ap()
    outmd_d = nc.dram_tensor("outmd_d", [1, TD * 128], F32,
                             kind="ExternalOutput").ap()

    with tile.TileContext(nc) as tc:
        with tc.sbuf_pool(name="const", bufs=1) as cp, \
             tc.sbuf_pool(name="work", bufs=1) as wp, \
             tc.psum_pool(name="ps", bufs=1) as pp:
            # ---- persistent tiles + input DMA ----
            xyf = cp.tile([66, XCOLS_], BF16, name="xyf")
            w1c0 = cp.tile([66, G], BF16, name="w1c0")
            w1h = cp.tile([128, 4 * G], BF16, name="w1h")
            w2 = cp.tile([128, 8 * G], BF16, name="w2")
            wmd = cp.tile([128, 4 * 64], BF16, name="wmd")
            b2r = cp.tile([1, G], BF16, name="b2r")
            bmd = cp.tile([1, 1], F32, name="bmd")
            ident = cp.tile([64, 64], F32, name="ident")
            outmd = cp.tile([1, TD * 128], F32, name="outmd")
            ones = cp.tile([1, 64], BF16, name="ones")
            nc.vector.memset(ones[:, :], 1.0)

            nc.sync.dma_start(xyf[:, :], xyf_d[:, :])
            nc.sync.dma_start(w1c0[:, :], w1c0_d[:, :])
            for k in range(4):
                nc.sync.dma_start(w1h[:, ts(k, G)], w1h_d[:, ts(k, G)])
            for k in range(8):
                nc.sync.dma_start(w2[:, ts(k, G)], w2_d[:, ts(k, G)])
            nc.sync.dma_start(wmd[:, :], wmd_d[:, :])
            nc.sync.dma_start(b2r[:, :], b2r_d[:, :])
            nc.sync.dma_start(bmd[:, :], bmd_d[:, :])
            nc.sync.dma_start(ident[:, :], id_d[:, :])

            # ---- state tiles ----
            c1 = cp.tile([64, H], F32, name="c1")
            c2 = cp.tile([64, H], F32, name="c2")
            nc.vector.memset(c1[:, :], 0.0)
            nc.vector.memset(c2[:, :], 0.0)

            def post_part(gp, c_state, h, sl, w, htag):
                """LSTM post for h columns sl (width w)."""
                i_s = wp.tile([64, w], F32, name=f"i{htag}", tag=f"i{htag}",
                              bufs=2)
                g_s = wp.tile([64, w], F32, name=f"g{htag}", tag=f"g{htag}",
                              bufs=2)
                f_s = wp.tile([64, w], F32, name=f"f{htag}", tag=f"f{htag}",
                              bufs=2)
                o_s = wp.tile([64, w], F32, name=f"o{htag}", tag=f"o{htag}",
                              bufs=2)
                nc.scalar.activation(i_s[:, :], gp[0][:, sl], Act.Sigmoid)
                nc.scalar.activation(g_s[:, :], gp[1][:, sl], Act.Tanh)
                nc.scalar.activation(f_s[:, :], gp[2][:, sl], Act.Sigmoid)
                nc.scalar.activation(o_s[:, :], gp[3][:, sl], Act.Sigmoid)
                t1 = wp.tile([64, w], F32, name=f"t1{htag}", tag=f"t1{htag}",
                             bufs=2)
                t2 = wp.tile([64, w], F32, name=f"t2{htag}", tag=f"t2{htag}",
                             bufs=2)
                nc.vector.tensor_mul(t1[:, :], i_s[:, :], g_s[:, :])
                nc.vector.tensor_mul(t2[:, :], f_s[:, :], c_state[:, sl])
                nc.vector.tensor_add(c_state[:, sl], t1[:, :], t2[:, :])
                tc_s = wp.tile([64, w], F32, name=f"tc{htag}",
                               tag=f"tc{htag}", bufs=2)
                nc.scalar.activation(tc_s[:, :], c_state[:, sl], Act.Tanh)
                nc.vector.tensor_mul(h[:, sl], o_s[:, :], tc_s[:, :])

            def transpose_part(trp, h, hT, kks):
                """h chunks kks -> trp psum -> hT bf16 chunk cols (DVE)."""
                for kk in kks:
                    nc.tensor.transpose(trp[:, ts(kk, 64)],
                                        h[:, ts(kk, 128)], ident[:, :])
                lo, hi = kks[0] * 64, (kks[-1] + 1) * 64
                nc.vector.tensor_copy(hT[:, lo:hi], trp[:, lo:hi])

            h1T_prev = None
            h2T_prev = None
            g1p_pend = None

            for t in range(tp):
                first = t == 0
                dec = t >= to - 1      # head/m-feedback steps
                # --- A: L1 xy finisher, N=256 halves (waits m-write in AR) ---
                if g1p_pend is None:
                    g1p = [pp.tile([64, H], F32, name=f"g1p{j}", tag="g1",
                                   bufs=4) for j in range(4)]
                else:
                    g1p = g1p_pend
                hs = True             # half-split posts everywhere
                if hs:
                    for half in range(2):
                        for j in range(4):
                            nc.tensor.matmul(
                                g1p[j][:, ts(half, HH)], xyf[0:66, ts(t, 64)],
                                w1c0[:, j * H + half * HH:
                                     j * H + (half + 1) * HH],
                                start=first, stop=True, skip_group_check=True)
                else:
                    for j in range(4):
                        nc.tensor.matmul(
                            g1p[j][:, :], xyf[0:66, ts(t, 64)],
                            w1c0[:, ts(j, H)],
                            start=first, stop=True, skip_group_check=True)
                # --- g2 group: bias (start) + B: h2 part, k-major ---
                g2p = [pp.tile([64, H], F32, name=f"g2p{j}", tag="g2",
                               bufs=3) for j in range(4)]
                for j in range(4):
                    nc.tensor.matmul(g2p[j][:, :], ones[:, :],
                                     b2r[:, ts(j, H)], start=True, stop=False,
                                     skip_group_check=True)
                if not first:
                    for k in range(4):
                        for j in range(4):
                            nc.tensor.matmul(
                                g2p[j][:, :], h2T_prev[:, ts(k, 64)],
                                w2[:, (4 + k) * G + j * H:(4 + k) * G + (j + 1) * H],
                                start=False, stop=False,
                                skip_group_check=True)
                # --- L1 post + transpose, per half ---
                h1 = wp.tile([64, H], F32, name="h1", tag="h1", bufs=2)
                h1T = wp.tile([128, 256], BF16, name="h1T", tag="h1T", bufs=2)
                trp1 = pp.tile([128, 256], F32, name="trp1", tag="small",
                               bufs=1)
                if hs:
                    for half in range(2):
                        post_part(g1p, c1, h1, ts(half, HH), HH, "1")
                        transpose_part(trp1, h1, h1T, (2 * half, 2 * half + 1))
                else:
                    post_part(g1p, c1, h1, slice(0, H), H, "1")
                    transpose_part(trp1, h1, h1T, (0, 1, 2, 3))
                # --- D: L2 h1 part, k-major; k=3 finisher split in halves ---
                for k in range(3):
                    for j in range(4):
                        nc.tensor.matmul(
                            g2p[j][:, :], h1T[:, ts(k, 64)],
                            w2[:, k * G + j * H:k * G + (j + 1) * H],
                            start=False, stop=False, skip_group_check=True)
                if hs:
                    for half in range(2):
                        for j in range(4):
                            nc.tensor.matmul(
                                g2p[j][:, ts(half, HH)], h1T[:, ts(3, 64)],
                                w2[:, 3 * G + j * H + half * HH:
                                   3 * G + j * H + (half + 1) * HH],
                                start=False, stop=True, skip_group_check=True)
                else:
                    for j in range(4):
                        nc.tensor.matmul(
                            g2p[j][:, :], h1T[:, ts(3, 64)],
                            w2[:, 3 * G + j * H:3 * G + (j + 1) * H],
                            start=False, stop=True, skip_group_check=True)
                # --- E: L1 h part for t+1, k-major (pipelined ahead) ---
                if t < tp - 1:
                    g1p_pend = [pp.tile([64, H], F32, name=f"g1pn{j}",
                                        tag="g1", bufs=4) for j in range(4)]
                    for k in range(4):
                        for j in range(4):
                            nc.tensor.matmul(
                                g1p_pend[j][:, :], h1T[:, ts(k, 64)],
                                w1h[:, k * G + j * H:k * G + (j + 1) * H],
                                start=(k == 0), stop=False,
                                skip_group_check=True)
                else:
                    g1p_pend = None
                # --- L2 post + transpose, per half ---
                h2 = wp.tile([64, H], F32, name="h2", tag="h2", bufs=2)
                h2T = wp.tile([128, 256], BF16, name="h2T", tag="h2T", bufs=2)
                trp2 = pp.tile([128, 256], F32, name="trp2", tag="small",
                               bufs=1)
                if hs:
                    for half in range(2):
                        post_part(g2p, c2, h2, ts(half, HH), HH, "2")
                        transpose_part(trp2, h2, h2T, (2 * half, 2 * half + 1))
                else:
                    post_part(g2p, c2, h2, slice(0, H), H, "2")
                    transpose_part(trp2, h2, h2T, (0, 1, 2, 3))
                # --- G: m/d head (AR feedback + staged outputs) ---
                if dec:
                    mdp = pp.tile([64, 64], F32, name="mdp", tag="small",
                                  bufs=1)
                    for k in range(4):
                        nc.tensor.matmul(mdp[:, :], wmd[:, ts(k, 64)],
                                         h2T[:, ts(k, 64)], start=(k == 0),
                                         stop=(k == 3),
                                         skip_group_check=True)
                    if t < tp - 1:
                        # m feedback -> feature row 0, slot t+1 (bf16)
                        nc.scalar.activation(xyf[0:1, ts(t + 1, 64)],
                                             mdp[0:1, :], Act.Identity,
                                             bias=bmd[0:1, 0:1], scale=1.0)
                    if t >= to:
                        s = t - to
                        # raw m/d rows -> staging tile (host adds bm/bd)
                        nc.scalar.activation(outmd[0:1, s * 128:s * 128 + 64],
                                             mdp[0:1, :], Act.Identity)
                        nc.scalar.activation(
                            outmd[0:1, s * 128 + 64:s * 128 + 128],
                            mdp[32:33, :], Act.Identity)
                h1T_prev, h2T_prev = h1T, h2T

            nc.sync.dma_start(outmd_d[:, :], outmd[:, :])

    if noload:
        drop_redundant_ldweights(nc)
    n = split_excess_waits(nc) if split_waits else 0
    return nc, n


_CACHE = {}


def _get_program():
    if "nc" not in _CACHE:
        _CACHE["nc"] = build_program()[0]
    return _CACHE["nc"]


def make_core_inputs(x, y, W1, b1, W2, b2, Wm, bm, Wd, bd, tp=TP, to=TO):
    """Host-side prep: returns (in_maps list of 8 dicts, scale [512])."""
    import ml_dtypes
    bf16 = ml_dtypes.bfloat16
    NSLOT_ = tp + 1
    XCOLS_ = NSLOT_ * B
    x = np.asarray(x, np.float32)
    y = np.asarray(y, np.float32)
    W1 = np.asarray(W1, np.float32)
    b1 = np.asarray(b1, np.float32)
    W2 = np.asarray(W2, np.float32)
    b2 = np.asarray(b2, np.float32)
    Wm = np.asarray(Wm, np.float32)
    bm = np.asarray(bm, np.float32)
    Wd = np.asarray(Wd, np.float32)
    bd = np.asarray(bd, np.float32)

    scale = 1.0 + np.mean(y[:, 0:to, 0], axis=1)       # [512]
    y_sc = y[:, 0:to, 0] / scale[:, None]              # [512, to]

    b1a = b1.copy()
    b1a[2 * H:3 * H] += 1.0                             # forget-gate +1
    b2a = b2.copy()
    b2a[2 * H:3 * H] += 1.0

    # row layout: 0 = y/m, 1:64 = x[0:63], 64 = ones (bias), 65 = x[63]
    w1c0 = np.empty((66, G), np.float32)
    w1c0[0] = W1[F]                                     # y/m weight row
    w1c0[1:64] = W1[0:F - 1]                            # x weight rows 0..62
    w1c0[64] = b1a                                      # bias row (ones input)
    w1c0[65] = W1[F - 1]                                # x weight row 63

    w1h = np.ascontiguousarray(
        W1[F + 1:].reshape(4, 128, G).transpose(1, 0, 2).reshape(128, 4 * G))
    w2 = np.ascontiguousarray(
        W2.reshape(8, 128, G).transpose(1, 0, 2).reshape(128, 8 * G))

    wmd = np.zeros((128, 4, 64), np.float32)
    wmd[:, :, 0] = Wm[:, 0].reshape(4, 128).T
    wmd[:, :, 32] = Wd[:, 0].reshape(4, 128).T
    wmd = np.ascontiguousarray(wmd.reshape(128, 4 * 64))

    b2row = np.ascontiguousarray(b2a.reshape(1, G))
    bmd = np.asarray(bm, np.float32).reshape(1, 1)
    identity = np.eye(64, dtype=np.float32)

    in_maps = []
    for c in range(NC):
        bs = slice(c * B, (c + 1) * B)
        xyf = np.zeros((66, NSLOT_, B), np.float32)
        xyf[0, 1:to, :] = y_sc[bs, 0:to - 1].T          # shifted y feed
        xt = x[bs].transpose(2, 1, 0)                   # [f, t, b]
        xyf[1:64, 0:tp, :] = xt[0:F - 1]                # x rows 0..62
        xyf[65, 0:tp, :] = xt[F - 1]                    # x row 63
        xyf[64, :, :] = 1.0                             # ones / bias row
        in_maps.append({
            "xyf_d": np.ascontiguousarray(
                xyf.reshape(66, XCOLS_)).astype(bf16),
            "w1c0_d": w1c0.astype(bf16), "w1h_d": w1h.astype(bf16),
            "w2_d": w2.astype(bf16), "wmd_d": wmd.astype(bf16),
            "b2r_d": b2row.astype(bf16), "bmd_d": bmd, "id_d": identity,
        })
    return in_maps, scale


def postprocess(results, scale, bm, bd, tp=TP, to=TO):
    """results: list of 8 dicts with outmd_d [tp-to, 128] -> [512, tp-to, 2]."""
    bm = float(np.asarray(bm).reshape(-1)[0])
    bd = float(np.asarray(bd).reshape(-1)[0])
    out = np.empty((B_FULL, tp - to, 2), np.float32)
    for c in range(NC):
        r = results[c]["outmd_d"].reshape(tp - to, 128)
        mean_tb = r[:, 0:64] + bm                       # [t, b]
        dpre_tb = r[:, 64:128] + bd
        bs = slice(c * B, (c + 1) * B)
        sc = scale[bs]
        out[bs, :, 0] = (mean_tb * sc[None, :]).T
        disp = np.logaddexp(dpre_tb, 0.0)               # softplus
        out[bs, :, 1] = (disp * np.sqrt(sc)[None, :]).T
    return out


def kernel(x, y, W1, b1, W2, b2, Wm, bm, Wd, bd):
    in_maps, scale = make_core_inputs(x, y, W1, b1, W2, b2, Wm, bm, Wd, bd)
    nc = _get_program()
    res = bass_utils.run_bass_kernel_spmd(nc, in_maps, core_ids=list(range(NC)))
    return postprocess(res.results, scale, bm, bd)


# revision 20
# speedup vs baseline: 2.8583x; 1.0118x over previous
"""DeepAR (2-layer LSTM, H=512) Trainium2 Bass kernel — v2.

Full-input contract: kernel(**inputs) takes the unsharded inputs from
setup_inputs() and returns the full [512, 64, 2] output.  Internally the
batch (512) is sharded 64-per-core across 8 NeuronCores (data parallel);
LSTM weights are replicated.

v2 changes over v1:
  - bf16 matmul operands (weights, features, transposed h); fp32 psum.
  - Phase A (L1 xy finisher) emitted FIRST each step so the h1 recurrence
    no longer chains through L2post+B. A is split into N=256 halves so
    the LSTM post can start per-half.
  - D/E are k-chunk-major so contraction chunks start as soon as each
    transposed h half lands; D's k=3 finisher is split into N=256 halves
    so L2post pipelines the same way.
  - L2 bias b2 enters PSUM via K=1 ones-row matmuls (start=True) instead
    of four DVE adds on the critical chain.
  - LSTM posts run per 256-col half: ACT reads gate psum directly,
    DVE does the c/h updates, ACT copies the transposed h into bf16.
  - Decode: m/d head outputs are copied from psum into an SBUF staging
    tile (host adds bm/bd); only the m feedback write touches xyf.
"""
import sys

sys.path.insert(0, "/opt/trn_rl_repo")

import numpy as np

import concourse.bass as bass
import concourse.mybir as mybir
from concourse import bass_utils, tile

F32 = mybir.dt.float32
BF16 = mybir.dt.bfloat16
FP8 = mybir.dt.float8e4
DR = mybir.MatmulPerfMode.DoubleRow
Act = mybir.ActivationFunctionType
USE_FP8 = False         # fp8e4m3 DoubleRow (fails 2e-2 accuracy gate)
WSCALE = 16.0           # weight pre-scale into fp8 normal range

B_FULL, TP, TO, F, H = 512, 192, 128, 64, 512
NC = 8
B = B_FULL // NC            # 64 per core
G = 4 * H                   # 2048 gate width
NSLOT = TP + 1              # 193 feature slots (slot t feeds step t)
XCOLS = NSLOT * B           # 12352
HH = H // 2                 # 256: half of hidden


def ts(i, n):
    return slice(i * n, (i + 1) * n)


def split_excess_waits(nc):
    """Walrus accepts only one sync-wait per hardware instruction. Hoist
    excess waits onto NoOps (same engine) inserted right before."""
    n = 0
    for f in nc.m.functions:
        for blk in f.blocks:
            out = []
            for inst in blk.instructions:
                si = inst.sync_info
                if si is not None and si.on_wait and len(si.on_wait) > 1:
                    waits = list(si.on_wait)
                    for j, w in enumerate(waits[:-1]):
                        nop = mybir.InstNoOp(
                            name=f"{inst.name}-wnop{j}", ins=[], outs=[])
                        nop.engine = inst.engine
                        nop.sync_info = mybir.SyncInfo(on_wait=[w], on_update=[])
                        out.append(nop)
                        n += 1
                    inst.sync_info = mybir.SyncInfo(
                        on_wait=[waits[-1]], on_update=list(si.on_update))
                out.append(inst)
            blk.instructions = out
    return n


def drop_redundant_ldweights(nc):
    """Remove InstLdweights that reload the stationary operand already in
    the PE array (identical AP as the previous retained load, no transpose
    in between). Waits/updates on dropped loads survive on a PE NoOp."""
    n = 0
    for f in nc.m.functions:
        for blk in f.blocks:
            out = []
            last_key = None
            for inst in blk.instructions:
                if isinstance(inst, mybir.InstLdweights):
                    w = inst.ins[0]
                    key = (getattr(w, "memref", None), w.offset, str(w.ap),
                           str(w.dtype), str(inst.perf_mode),
                           str(inst.tile_position))
                    if key == last_key:
                        si = inst.sync_info
                        if si is not None and (si.on_wait or si.on_update):
                            nop = mybir.InstNoOp(
                                name=f"{inst.name}-ldwnop", ins=[], outs=[])
                            nop.engine = inst.engine
                            nop.sync_info = si
                            out.append(nop)
                        n += 1
                        continue
                    last_key = key
                elif isinstance(inst, mybir.InstMatmult):
                    if inst.is_transpose:
                        last_key = None
                out.append(inst)
            blk.instructions = out
    return n


def build_program(tp=TP, to=TO, split_waits=True, noload=True):
    NSLOT_ = tp + 1
    XCOLS_ = NSLOT_ * B
    TD = tp - to                # decode steps (64)
    nc = bass.Bass("TRN2", target_bir_lowering=False, debug=False,
                   num_devices=NC)

    xyf_d = nc.dram_tensor("xyf_d", [66, XCOLS_], BF16, kind="ExternalInput").ap()
    w1c0_d = nc.dram_tensor("w1c0_d", [66, G], BF16, kind="ExternalInput").ap()
    hdt = FP8 if USE_FP8 else BF16
    w1h_d = nc.dram_tensor("w1h_d", [128, 4, G], hdt, kind="ExternalInput").ap()
    w2_d = nc.dram_tensor("w2_d", [128, 8, G], hdt, kind="ExternalInput").ap()
    wmd_d = nc.dram_tensor("wmd_d", [128, 4 * 64], BF16, kind="ExternalInput").ap()
    b2r_d = nc.dram_tensor("b2r_d", [1, G], BF16, kind="ExternalInput").ap()
    bmd_d = nc.dram_tensor("bmd_d", [1, 1], F32, kind="ExternalInput").ap()
    id_d = nc.dram_tensor("id_d", [64, 64], F32, kind="ExternalInput").ap()
    outmd_d = nc.dram_tensor("outmd_d", [1, TD * 128], F32,
                             kind="ExternalOutput").ap()

    with tile.TileContext(nc) as tc:
        with tc.sbuf_pool(name="const", bufs=1) as cp, \
             tc.sbuf_pool(name="work", bufs=1) as wp, \
             tc.psum_pool(name="ps", bufs=1) as pp:
            # ---- persistent tiles + input DMA ----
            xyf = cp.tile([66, XCOLS_], BF16, name="xyf")
            w1c0 = cp.tile([66, G], BF16, name="w1c0")
            w1h = cp.tile([128, 4, G], hdt, name="w1h")
            w2 = cp.tile([128, 8, G], hdt, name="w2")
            wmd = cp.tile([128, 4 * 64], BF16, name="wmd")
            b2r = cp.tile([1, G], BF16, name="b2r")
            bmd = cp.tile([1, 1], F32, name="bmd")
            ident = cp.tile([64, 64], F32, name="ident")
            outmd = cp.tile([1, TD * 128], F32, name="outmd")
            ones = cp.tile([1, 64], BF16, name="ones")
            nc.vector.memset(ones[:, :], 1.0)

            nc.sync.dma_start(xyf[:, :], xyf_d[:, :])
            nc.sync.dma_start(w1c0[:, :], w1c0_d[:, :])
            for k in range(4):
                nc.sync.dma_start(w1h[:, k:k + 1, :], w1h_d[:, k:k + 1, :])
            for k in range(8):
                nc.sync.dma_start(w2[:, k:k + 1, :], w2_d[:, k:k + 1, :])
            nc.sync.dma_start(wmd[:, :], wmd_d[:, :])
            nc.sync.dma_start(b2r[:, :], b2r_d[:, :])
            nc.sync.dma_start(bmd[:, :], bmd_d[:, :])
            nc.sync.dma_start(ident[:, :], id_d[:, :])

            # ---- state tiles ----
            c1 = cp.tile([64, H], F32, name="c1")
            c2 = cp.tile([64, H], F32, name="c2")
            nc.vector.memset(c1[:, :], 0.0)
            nc.vector.memset(c2[:, :], 0.0)

            def gates_part(gp, sl, w, htag):
                """Gate activations for columns sl (ACT, reads psum)."""
                i_s = wp.tile([64, w], F32, name=f"i{htag}", tag=f"i{htag}",
                              bufs=2)
                g_s = wp.tile([64, w], F32, name=f"g{htag}", tag=f"g{htag}",
                              bufs=2)
                f_s = wp.tile([64, w], F32, name=f"f{htag}", tag=f"f{htag}",
                              bufs=2)
                o_s = wp.tile([64, w], F32, name=f"o{htag}", tag=f"o{htag}",
                              bufs=2)
                gs = 1.0 / WSCALE if USE_FP8 else 1.0
                nc.scalar.activation(i_s[:, :], gp[0][:, sl], Act.Sigmoid,
                                     scale=gs)
                nc.scalar.activation(g_s[:, :], gp[1][:, sl], Act.Tanh,
                                     scale=gs)
                nc.scalar.activation(f_s[:, :], gp[2][:, sl], Act.Sigmoid,
                                     scale=gs)
                nc.scalar.activation(o_s[:, :], gp[3][:, sl], Act.Sigmoid,
                                     scale=gs)
                return i_s, g_s, f_s, o_s

            def tail_part(acts, c_state, h, sl, w, htag):
                """c/h update for columns sl (DVE + ACT tanh + GPSIMD mul)."""
                i_s, g_s, f_s, o_s = acts
                t1 = wp.tile([64, w], F32, name=f"t1{htag}", tag=f"t1{htag}",
                             bufs=2)
                t2 = wp.tile([64, w], F32, name=f"t2{htag}", tag=f"t2{htag}",
                             bufs=2)
                nc.vector.tensor_mul(t1[:, :], i_s[:, :], g_s[:, :])
                nc.vector.tensor_mul(t2[:, :], f_s[:, :], c_state[:, sl])
                nc.vector.tensor_add(c_state[:, sl], t1[:, :], t2[:, :])
                tc_s = wp.tile([64, w], F32, name=f"tc{htag}",
                               tag=f"tc{htag}", bufs=2)
                nc.scalar.activation(tc_s[:, :], c_state[:, sl], Act.Tanh)
                nc.vector.tensor_mul(h[:, sl], o_s[:, :], tc_s[:, :])

            def transpose_part(trp, h, hT, kks, hTb=None):
                """h chunks kks -> trp psum -> hT chunk cols (DVE); optional
                extra bf16 copy (ACT) for the m/d head."""
                for kk in kks:
                    nc.tensor.transpose(trp[:, ts(kk, 64)],
                                        h[:, ts(kk, 128)], ident[:, :])
                lo, hi = kks[0] * 64, (kks[-1] + 1) * 64
                if USE_FP8:
                    nc.vector.tensor_copy(hT[:, kks[0]:kks[-1] + 1, :],
                                          trp[:, lo:hi])
                else:
                    nc.vector.tensor_copy(hT[:, lo:hi], trp[:, lo:hi])
                if hTb is not None:
                    nc.scalar.activation(hTb[:, lo:hi], trp[:, lo:hi],
                                         Act.Identity)

            h1T_prev = None
            h2T_prev = None
            g1p_pend = None

            for t in range(tp):
                first = t == 0
                dec = t >= to - 1      # head/m-feedback steps
                # --- A: L1 xy finisher, N=256 halves (waits m-write in AR) ---
                if g1p_pend is None:
                    g1p = [pp.tile([64, H], F32, name=f"g1p{j}", tag="g1",
                                   bufs=4) for j in range(4)]
                else:
                    g1p = g1p_pend
                hs = True             # half-split posts everywhere
                if hs:
                    for half in range(2):
                        for j in range(4):
                            nc.tensor.matmul(
                                g1p[j][:, ts(half, HH)], xyf[0:66, ts(t, 64)],
                                w1c0[:, j * H + half * HH:
                                     j * H + (half + 1) * HH],
                                start=first, stop=True, skip_group_check=True)
                else:
                    for j in range(4):
                        nc.tensor.matmul(
                            g1p[j][:, :], xyf[0:66, ts(t, 64)],
                            w1c0[:, ts(j, H)],
                            start=first, stop=True, skip_group_check=True)
                # --- g2 group: bias (start) + B: h2 part, k-major ---
                g2p = [pp.tile([64, H], F32, name=f"g2p{j}", tag="g2",
                               bufs=3) for j in range(4)]
                for j in range(4):
                    nc.tensor.matmul(g2p[j][:, :], ones[:, :],
                                     b2r[:, ts(j, H)], start=True, stop=False,
                                     skip_group_check=True)
                if not first:
                    if USE_FP8:
                        for P in range(2):
                            for j in range(4):
                                nc.tensor.matmul(
                                    g2p[j][:, :],
                                    h2T_prev[:, 2 * P:2 * P + 2, :],
                                    w2[:, 4 + 2 * P:4 + 2 * P + 2, ts(j, H)],
                                    start=False, stop=False, perf_mode=DR,
                                    skip_group_check=True)
                    else:
                        for k in range(4):
                            for j in range(4):
                                nc.tensor.matmul(
                                    g2p[j][:, :], h2T_prev[:, ts(k, 64)],
                                    w2[:, 4 + k:5 + k, ts(j, H)],
                                    start=False, stop=False,
                                    skip_group_check=True)
                # --- L1 post + transpose, per half ---
                h1 = wp.tile([64, H], F32, name="h1", tag="h1", bufs=2)
                if USE_FP8:
                    h1T = wp.tile([128, 4, 64], FP8, name="h1T", tag="h1T",
                                  bufs=2)
                else:
                    h1T = wp.tile([128, 256], BF16, name="h1T", tag="h1T",
                                  bufs=2)
                trp1 = pp.tile([128, 256], F32, name="trp1", tag="small",
                               bufs=1)
                if hs:
                    acts0 = gates_part(g1p, ts(0, HH), HH, "1")
                    tail_part(acts0, c1, h1, ts(0, HH), HH, "1")
                    transpose_part(trp1, h1, h1T, (0, 1))
                    acts1 = gates_part(g1p, ts(1, HH), HH, "1")
                    tail_part(acts1, c1, h1, ts(1, HH), HH, "1")
                    transpose_part(trp1, h1, h1T, (2, 3))
                else:
                    acts0 = gates_part(g1p, slice(0, H), H, "1")
                    tail_part(acts0, c1, h1, slice(0, H), H, "1")
                    transpose_part(trp1, h1, h1T, (0, 1, 2, 3))
                # --- D: L2 h1 part, k-major; k=3 finisher split in halves ---
                if USE_FP8:
                    for j in range(4):
                        nc.tensor.matmul(
                            g2p[j][:, :], h1T[:, 0:2, :],
                            w2[:, 0:2, ts(j, H)],
                            start=False, stop=False, perf_mode=DR,
                            skip_group_check=True)
                    if hs:
                        for half in range(2):
                            for j in range(4):
                                nc.tensor.matmul(
                                    g2p[j][:, ts(half, HH)], h1T[:, 2:4, :],
                                    w2[:, 2:4, j * H + half * HH:
                                       j * H + (half + 1) * HH],
                                    start=False, stop=True, perf_mode=DR,
                                    skip_group_check=True)
                    else:
                        for j in range(4):
                            nc.tensor.matmul(
                                g2p[j][:, :], h1T[:, 2:4, :],
                                w2[:, 2:4, ts(j, H)],
                                start=False, stop=True, perf_mode=DR,
                                skip_group_check=True)
                else:
                    for k in range(3):
                        for j in range(4):
                            nc.tensor.matmul(
                                g2p[j][:, :], h1T[:, ts(k, 64)],
                                w2[:, k:k + 1, ts(j, H)],
                                start=False, stop=False, skip_group_check=True)
                    if hs:
                        for half in range(2):
                            for j in range(4):
                                nc.tensor.matmul(
                                    g2p[j][:, ts(half, HH)], h1T[:, ts(3, 64)],
                                    w2[:, 3:4, j * H + half * HH:
                                       j * H + (half + 1) * HH],
                                    start=False, stop=True,
                                    skip_group_check=True)
                    else:
                        for j in range(4):
                            nc.tensor.matmul(
                                g2p[j][:, :], h1T[:, ts(3, 64)],
                                w2[:, 3:4, ts(j, H)],
                                start=False, stop=True, skip_group_check=True)
                # --- E: L1 h part for t+1, k-major (pipelined ahead) ---
                if t < tp - 1:
                    g1p_pend = [pp.tile([64, H], F32, name=f"g1pn{j}",
                                        tag="g1", bufs=4) for j in range(4)]
                    if USE_FP8:
                        for P in range(2):
                            for j in range(4):
                                nc.tensor.matmul(
                                    g1p_pend[j][:, :], h1T[:, 2 * P:2 * P + 2, :],
                                    w1h[:, 2 * P:2 * P + 2, ts(j, H)],
                                    start=(P == 0), stop=False, perf_mode=DR,
                                    skip_group_check=True)
                    else:
                        for k in range(4):
                            for j in range(4):
                                nc.tensor.matmul(
                                    g1p_pend[j][:, :], h1T[:, ts(k, 64)],
                                    w1h[:, k:k + 1, ts(j, H)],
                                    start=(k == 0), stop=False,
                                    skip_group_check=True)
                else:
                    g1p_pend = None
                # --- L2 post + transpose, per half ---
                h2 = wp.tile([64, H], F32, name="h2", tag="h2", bufs=2)
                if USE_FP8:
                    h2T = wp.tile([128, 4, 64], FP8, name="h2T", tag="h2T",
                                  bufs=2)
                    h2Tb = (wp.tile([128, 256], BF16, name="h2Tb", tag="h2Tb",
                                    bufs=2) if dec else None)
                else:
                    h2T = wp.tile([128, 256], BF16, name="h2T", tag="h2T",
                                  bufs=2)
                    h2Tb = None
                trp2 = pp.tile([128, 256], F32, name="trp2", tag="small",
                               bufs=1)
                if hs:
                    acts0 = gates_part(g2p, ts(0, HH), HH, "2")
                    tail_part(acts0, c2, h2, ts(0, HH), HH, "2")
                    transpose_part(trp2, h2, h2T, (0, 1), hTb=h2Tb)
                    acts1 = gates_part(g2p, ts(1, HH), HH, "2")
                    tail_part(acts1, c2, h2, ts(1, HH), HH, "2")
                    transpose_part(trp2, h2, h2T, (2, 3), hTb=h2Tb)
                else:
                    acts0 = gates_part(g2p, slice(0, H), H, "2")
                    tail_part(acts0, c2, h2, slice(0, H), H, "2")
                    transpose_part(trp2, h2, h2T, (0, 1, 2, 3), hTb=h2Tb)
                # --- G: m/d head (AR feedback + staged outputs) ---
                if dec:
                    mdp = pp.tile([64, 64], F32, name="mdp", tag="small",
                                  bufs=1)
                    hsrc = h2Tb if USE_FP8 else h2T
                    for k in range(4):
                        nc.tensor.matmul(mdp[:, :], wmd[:, ts(k, 64)],
                                         hsrc[:, ts(k, 64)], start=(k == 0),
                                         stop=(k == 3),
                                         skip_group_check=True)
                    if t < tp - 1:
                        # m feedback -> feature row 0, slot t+1 (bf16)
                        nc.scalar.activation(xyf[0:1, ts(t + 1, 64)],
                                             mdp[0:1, :], Act.Identity,
                                             bias=bmd[0:1, 0:1], scale=1.0)
                    if t >= to:
                        s = t - to
                        # raw m/d rows -> staging tile (host adds bm/bd)
                        nc.scalar.activation(outmd[0:1, s * 128:s * 128 + 64],
                                             mdp[0:1, :], Act.Identity)
                        nc.scalar.activation(
                            outmd[0:1, s * 128 + 64:s * 128 + 128],
                            mdp[32:33, :], Act.Identity)
                h1T_prev, h2T_prev = h1T, h2T

            nc.sync.dma_start(outmd_d[:, :], outmd[:, :])

    if noload:
        drop_redundant_ldweights(nc)
    n = split_excess_waits(nc) if split_waits else 0
    return nc, n


_CACHE = {}


def _get_program():
    if "nc" not in _CACHE:
        _CACHE["nc"] = build_program()[0]
    return _CACHE["nc"]


def make_core_inputs(x, y, W1, b1, W2, b2, Wm, bm, Wd, bd, tp=TP, to=TO):
    """Host-side prep: returns (in_maps list of 8 dicts, scale [512])."""
    import ml_dtypes
    bf16 = ml_dtypes.bfloat16
    hdt_np = mybir.dt.np(FP8 if USE_FP8 else BF16)
    ws = WSCALE if USE_FP8 else 1.0
    NSLOT_ = tp + 1
    XCOLS_ = NSLOT_ * B
    x = np.asarray(x, np.float32)
    y = np.asarray(y, np.float32)
    W1 = np.asarray(W1, np.float32)
    b1 = np.asarray(b1, np.float32)
    W2 = np.asarray(W2, np.float32)
    b2 = np.asarray(b2, np.float32)
    Wm = np.asarray(Wm, np.float32)
    bm = np.asarray(bm, np.float32)
    Wd = np.asarray(Wd, np.float32)
    bd = np.asarray(bd, np.float32)

    scale = 1.0 + np.mean(y[:, 0:to, 0], axis=1)       # [512]
    y_sc = y[:, 0:to, 0] / scale[:, None]              # [512, to]

    b1a = b1.copy()
    b1a[2 * H:3 * H] += 1.0                             # forget-gate +1
    b2a = b2.copy()
    b2a[2 * H:3 * H] += 1.0

    # row layout: 0 = y/m, 1:64 = x[0:63], 64 = ones (bias), 65 = x[63]
    w1c0 = np.empty((66, G), np.float32)
    w1c0[0] = W1[F]                                     # y/m weight row
    w1c0[1:64] = W1[0:F - 1]                            # x weight rows 0..62
    w1c0[64] = b1a                                      # bias row (ones input)
    w1c0[65] = W1[F - 1]                                # x weight row 63

    w1h = np.ascontiguousarray(
        W1[F + 1:].reshape(4, 128, G).transpose(1, 0, 2)) * ws
    w2 = np.ascontiguousarray(
        W2.reshape(8, 128, G).transpose(1, 0, 2)) * ws

    wmd = np.zeros((128, 4, 64), np.float32)
    wmd[:, :, 0] = Wm[:, 0].reshape(4, 128).T
    wmd[:, :, 32] = Wd[:, 0].reshape(4, 128).T
    wmd = np.ascontiguousarray(wmd.reshape(128, 4 * 64))

    b2row = np.ascontiguousarray(b2a.reshape(1, G))
    bmd = np.asarray(bm, np.float32).reshape(1, 1)
    identity = np.eye(64, dtype=np.float32)

    in_maps = []
    for c in range(NC):
        bs = slice(c * B, (c + 1) * B)
        xyf = np.zeros((66, NSLOT_, B), np.float32)
        xyf[0, 1:to, :] = y_sc[bs, 0:to - 1].T          # shifted y feed
        xt = x[bs].transpose(2, 1, 0)                   # [f, t, b]
        xyf[1:64, 0:tp, :] = xt[0:F - 1]                # x rows 0..62
        xyf[65, 0:tp, :] = xt[F - 1]                    # x row 63
        xyf[64, :, :] = 1.0                             # ones / bias row
        in_maps.append({
            "xyf_d": np.ascontiguousarray(
                xyf.reshape(66, XCOLS_)).astype(bf16),
            "w1c0_d": (w1c0 * ws).astype(bf16),
            "w1h_d": w1h.astype(hdt_np),
            "w2_d": w2.astype(hdt_np), "wmd_d": wmd.astype(bf16),
            "b2r_d": (b2row * ws).astype(bf16), "bmd_d": bmd,
            "id_d": identity,
        })
    return in_maps, scale


def postprocess(results, scale, bm, bd, tp=TP, to=TO):
    """results: list of 8 dicts with outmd_d [tp-to, 128] -> [512, tp-to, 2]."""
    bm = float(np.asarray(bm).reshape(-1)[0])
    bd = float(np.asarray(bd).reshape(-1)[0])
    out = np.empty((B_FULL, tp - to, 2), np.float32)
    for c in range(NC):
        r = results[c]["outmd_d"].reshape(tp - to, 128)
        mean_tb = r[:, 0:64] + bm                       # [t, b]
        dpre_tb = r[:, 64:128] + bd
        bs = slice(c * B, (c + 1) * B)
        sc = scale[bs]
        out[bs, :, 0] = (mean_tb * sc[None, :]).T
        disp = np.logaddexp(dpre_tb, 0.0)               # softplus
        out[bs, :, 1] = (disp * np.sqrt(sc)[None, :]).T
    return out


def kernel(x, y, W1, b1, W2, b2, Wm, bm, Wd, bd):
    in_maps, scale = make_core_inputs(x, y, W1, b1, W2, b2, Wm, bm, Wd, bd)
    nc = _get_program()
    res = bass_utils.run_bass_kernel_spmd(nc, in_maps, core_ids=list(range(NC)))
    return postprocess(res.results, scale, bm, bd)
